# revision 22
# baseline (speedup 1.0000x reference)
"""Trainium2 Bass kernel for GQA attention with RoPE, causal mask, and
attention sinks (nn_Attention_65094524338392).

Sharding: tensor-parallel by heads across 8 NeuronCores. Core c owns query
heads 4c..4c+3 and kv-head c (NREP=4). Each core computes QKV projections
over the full sequence for its heads and flash-style causal attention.

v3 design:
- x is transposed on the HOST (free) and streamed as one contiguous DMA
  per 512-token block (no device DMA-transposes). Late-needed loads (woT,
  x blocks 1+) are data-gated so they don't steal HBM bandwidth from the
  startup-critical wqkvT + x-block-0 transfers.
- QKV/rope/attention are interleaved per 512-block: B0 C0 B1 C1 ...
- The head->sequence redistribution is split into 8 AllToAlls (one per
  block and head-pair; sequence ownership interleaved: core c owns rows
  512*s + 64c..+64 of each block s), each overlapped with later compute;
  the output projection overlaps the last ones.
- Scores for the diagonal 128-tiles are trimmed/compacted; the causal
  micro-mask is one [128,128] triangle applied with a single 3D-AP
  vector multiply per k-tile pair.
- Dummy PE warm bursts at attention-group boundaries keep the HAM DVFS
  clock high (same trick as the previous kernel generation).

Math note: the sink scaling folds into the softmax normalizer:
    out = (sum_k exp(s_k) v_k) / (sum_k exp(s_k) + exp(sink))
so no logs/sigmoids are needed on device, and because |s| <= ~40 no
max-subtraction is needed for exp stability in fp32 accumulation.
"""

import os
import sys

sys.path.insert(0, "/opt/trn_rl_repo")

import ml_dtypes
import numpy as np

import concourse.bass as bass
import concourse.mybir as mybir
import concourse.tile as tile
from concourse import bacc
from concourse.bass_utils import run_bass_kernel_spmd

# Problem shapes
B, S, DIM = 1, 2048, 2048
NH, NKV, HD = 32, 8, 64
NREP = NH // NKV
SCALE = 1.0 / float(np.sqrt(HD))
NCORES = 8
HPC = NH // NCORES            # query heads per core (4)
QKV = HPC * HD + 2 * HD       # fused qkv output dim per core (384)
QW = HPC * HD                 # query width per core (256)
SB = 512                      # seq block
NSB = S // SB                 # 4
NT = S // 128                 # 16 seq tiles
ND = DIM // 128               # 16 contraction tiles
MYS = S // NCORES             # output rows per core (256)
CH = MYS // NSB               # rows per core per block (64)

F32 = mybir.dt.float32
BF16 = mybir.dt.bfloat16

_cache = {}

last_exec_time_ns = None
last_result = None


def _install_ntff_shim():
    """Register the NTFF profile hook so trace=True yields exec_time_ns."""
    import types
    if "antenv.axon_hooks" in sys.modules:
        return
    import antenv
    mod = types.ModuleType("antenv.axon_hooks")
    mod._hook = None
    mod.set_axon_ntff_profile_hook = lambda h: setattr(mod, "_hook", h)
    mod.get_axon_ntff_profile_hook = lambda: mod._hook
    sys.modules["antenv.axon_hooks"] = mod
    antenv.axon_hooks = mod
    from trn_agent_boot.trn_boot import _ntff_profile_via_ctypes
    hook = _ntff_profile_via_ctypes("/opt/axon/libaxon_pjrt.so")
    if hook is not None:
        mod._hook = hook


def _build():
    nc = bacc.Bacc("TRN2", target_bir_lowering=False, debug=False,
                   num_devices=NCORES)

    ident_e = nc.declare_dram_parameter("ident", [128, 128], BF16, isOutput=False)
    qkvb_e = nc.declare_dram_parameter("qkvb", [1, QKV], BF16, isOutput=False)
    cosd_e = nc.declare_dram_parameter("cosd", [128, NT * HD], F32, isOutput=False)
    nsin_e = nc.declare_dram_parameter("nsin", [128, NT * HD // 2], F32, isOutput=False)
    psin_e = nc.declare_dram_parameter("psin", [128, NT * HD // 2], F32, isOutput=False)
    sinks_e = nc.declare_dram_parameter("sinks4", [1, HPC], F32, isOutput=False)
    wob_e = nc.declare_dram_parameter("wob", [1, DIM], BF16, isOutput=False)
    tri_e = nc.declare_dram_parameter("tri", [128, 128], BF16, isOutput=False)
    wqkvT_e = nc.declare_dram_parameter("wqkvT", [128, ND * QKV], BF16, isOutput=False)
    xT_e = nc.declare_dram_parameter("xT", [128, NSB * ND * SB], BF16, isOutput=False)
    woT_e = nc.declare_dram_parameter("woT", [128, ND * DIM], BF16, isOutput=False)
    out_e = nc.declare_dram_parameter("out", [MYS, DIM], F32, isOutput=True)

    xT_v = xT_e[:].rearrange("p (s d c) -> p s d c", s=NSB, d=ND)

    with tile.TileContext(nc) as tc:
        with tc.tile_pool(name="const", bufs=1) as cp, \
             tc.tile_pool(name="xT", bufs=2) as xtp, \
             tc.tile_pool(name="rope", bufs=3) as rp, \
             tc.tile_pool(name="qkr", bufs=5) as qkrp, \
             tc.tile_pool(name="pt", bufs=4) as ptp, \
             tc.tile_pool(name="ep", bufs=2) as epp, \
             tc.tile_pool(name="fin", bufs=3) as fnp, \
             tc.tile_pool(name="dram", bufs=1, space="DRAM") as dp, \
             tc.tile_pool(name="pp", bufs=2, space="PSUM") as pp:

            # ---- startup ----
            # gpsimd: memsets first (instant; unblock warmup + bias rows)
            ones_sb = cp.tile([1, 128], BF16)
            nc.gpsimd.memset(ones_sb[:], 1.0)
            ones512 = cp.tile([1, 512], BF16)
            nc.gpsimd.memset(ones512[:], 1.0)
            warm_sb = cp.tile([128, 512], BF16)
            nc.gpsimd.memset(warm_sb[:], 0.0)
            # sync queue: wqkvT then x^T block 0 (the startup-critical loads)
            wqkvT_sb = cp.tile([128, ND, QKV], BF16)
            nc.sync.dma_start(wqkvT_sb[:], wqkvT_e[:].rearrange(
                "p (o f) -> p o f", o=ND))
            xts = [None] * NSB
            xts[0] = xtp.tile([128, ND, SB], BF16, tag="xT", name="xT0")
            nc.sync.dma_start(xts[0][:], xT_v[:, 0])
            # scalar queue (HWDGE): tiny consts + rope tables
            ident_sb = cp.tile([128, 128], BF16)
            nc.scalar.dma_start(ident_sb[:], ident_e[:])
            qkvb_sb = cp.tile([1, QKV], BF16)
            nc.scalar.dma_start(qkvb_sb[:], qkvb_e[:])
            tri_sb = cp.tile([128, 128], BF16)
            nc.scalar.dma_start(tri_sb[:], tri_e[:])
            sinks_sb = cp.tile([1, HPC], F32)
            nc.scalar.dma_start(sinks_sb[:], sinks_e[:])
            wob_sb = cp.tile([1, DIM], BF16)
            nc.scalar.dma_start(wob_sb[:], wob_e[:])
            # warm the ACT exp table during startup
            es_sb = cp.tile([1, HPC], F32)
            nc.scalar.activation(es_sb[:], sinks_sb[:],
                                 mybir.ActivationFunctionType.Exp)
            cos_sb = cp.tile([128, NT, HD], F32)
            nc.scalar.dma_start(cos_sb[:], cosd_e[:].rearrange(
                "p (o f) -> p o f", o=NT))
            nsin_sb = cp.tile([128, NT, HD // 2], F32)
            nc.scalar.dma_start(nsin_sb[:], nsin_e[:].rearrange(
                "p (o f) -> p o f", o=NT))
            psin_sb = cp.tile([128, NT, HD // 2], F32)
            nc.scalar.dma_start(psin_sb[:], psin_e[:].rearrange(
                "p (o f) -> p o f", o=NT))
            # x^T block 1: gated on the rope tables' arrival (the last
            # startup-critical load) so its 2MB doesn't steal HBM bandwidth
            # from them; B1 needs it much later
            xts[1] = xtp.tile([128, ND, SB], BF16, tag="xT", name="xT1")
            nc.gpsimd.tensor_copy(xts[1][0:1, 0, 0:2],
                                  psin_sb[0:1, NT - 1, HD // 2 - 2:HD // 2])
            nc.sync.dma_start(xts[1][:], xT_v[:, 1])

            warm2_sb = cp.tile([128, 512], BF16)
            nc.gpsimd.memset(warm2_sb[:], 0.0)

            def warm_burst(n, name, src_sb=None):
                ws = warm_sb if src_sb is None else src_sb
                w = pp.tile([128, 512], F32, tag="acc", bufs=2, name=name)
                for _ in range(n):
                    nc.tensor.matmul(w[:, 0:512], ws[:, 0:128],
                                     ws[:], start=True, stop=True)

            # PE warm-up during the initial input DMAs (HAM clock release)
            warm_burst(16, "warm0")
            woT_sb = cp.tile([128, ND, DIM], BF16)

            # persistent activations
            qP = [cp.tile([128, S], BF16, name=f"qP{g}") for g in range(HPC // 2)]
            kTd = cp.tile([128, S], BF16)
            v_sb = cp.tile([128, NT, HD + 1], BF16)
            nc.gpsimd.memset(v_sb[:, :, HD:HD + 1], 1.0)
            oT = [cp.tile([HD, S], BF16, name=f"oT{h}") for h in range(HPC)]
            ag_sb = cp.tile([128, ND, MYS], BF16)

            a2a_in = [dp.tile([NH * HD, CH], BF16, name=f"a2ain{s}")
                      for s in range(NSB)]
            a2a_out = [tc.tile([NH * HD, CH], BF16, space="DRAM",
                               addr_space="Shared", name=f"a2aout{s}")[0]
                       for s in range(NSB)]

            state = {"pt0": None}

            def emit_B(s):
                xT_s = xts[s]
                qkr_tiles = []
                for tt in range(4):
                    t = 4 * s + tt
                    acc = pp.tile([128, 512], F32, tag="acc", bufs=2,
                                  name=f"acc{t}")
                    for d in range(ND):
                        nc.tensor.matmul(acc[:, 0:QKV],
                                         xT_s[:, d, tt * 128:(tt + 1) * 128],
                                         wqkvT_sb[:, d, :],
                                         start=(d == 0), stop=False)
                    nc.tensor.matmul(acc[:, 0:QKV], ones_sb[0:1, :], qkvb_sb[:],
                                     start=False, stop=True)

                    # rope on q and k (free-dim halves; 5 = 4q + 1k groups)
                    W = QW + HD  # 320
                    tmp = rp.tile([128, W], F32, tag="tmp")
                    qkr = qkrp.tile([128, W], BF16, tag="qkr")
                    acc5 = acc[:, 0:W].rearrange("p (h x) -> p h x", x=HD)
                    tmp5 = tmp[:].rearrange("p (h x) -> p h x", x=HD)
                    nc.vector.tensor_tensor(
                        tmp5[:, :, 0:HD // 2], acc5[:, :, HD // 2:HD],
                        nsin_sb[:, t:t + 1, :].to_broadcast([128, 5, HD // 2]),
                        mybir.AluOpType.mult)
                    nc.vector.tensor_tensor(
                        tmp5[:, :, HD // 2:HD], acc5[:, :, 0:HD // 2],
                        psin_sb[:, t:t + 1, :].to_broadcast([128, 5, HD // 2]),
                        mybir.AluOpType.mult)
                    nc.vector.tensor_tensor(
                        qkr[:], acc[:, 0:W],
                        cos_sb[:, t:t + 1, :].to_broadcast([128, 5, HD]),
                        mybir.AluOpType.mult)
                    nc.vector.tensor_tensor(qkr[:], qkr[:], tmp[:],
                                            mybir.AluOpType.add)
                    qkr_tiles.append(qkr)
                    # v: plain copy (cast to bf16)
                    nc.scalar.copy(v_sb[:, t, 0:HD], acc[:, QW + HD:QKV])

                # transpose rope'd q/k into [hd, seq] layout
                sl = slice(s * SB, (s + 1) * SB)
                for h in range(HPC + 1):
                    tq_ps = pp.tile([HD, 512], BF16, tag="sc", bufs=2,
                                    name=f"tq{s}_{h}")
                    for tt in range(4):
                        nc.tensor.transpose(
                            tq_ps[:, tt * 128:(tt + 1) * 128],
                            qkr_tiles[tt][:, h * HD:(h + 1) * HD], ident_sb[:])
                    if h < HPC:
                        dst = qP[h // 2][(h % 2) * HD:(h % 2) * HD + HD, sl]
                        if h % 2 == 0:
                            nc.vector.tensor_copy(dst, tq_ps[:])
                        else:
                            nc.scalar.copy(dst, tq_ps[:])
                    else:
                        nc.scalar.copy(kTd[0:HD, sl], tq_ps[:])
                        nc.vector.tensor_copy(kTd[HD:2 * HD, sl], tq_ps[:])

            def emit_C_group(s, g, pvs):
                n_kt = 4 * (s + 1)
                pend = []

                def flush_pv():
                    for (pts_, segs_, z_) in pend:
                        for (i, off, ln, q0) in segs_:
                            nc.tensor.matmul(
                                pvs[z_][:, q0:q0 + ln],
                                v_sb[:, i, :], pts_[:, off:off + ln],
                                start=(i == 0), stop=(i == n_kt - 1),
                                skip_group_check=True)
                    pend.clear()

                for p in range(n_kt // 2):
                    i0, i1 = 2 * p, 2 * p + 1
                    diag = (i0 >= 4 * s)
                    if diag:
                        d0, d1 = i0 - 4 * s, i1 - 4 * s
                        l0, l1 = 512 - 128 * d0, 512 - 128 * d1
                    else:
                        l0 = l1 = 512
                    # segs: (ktile, col offset in sc/pt, len, q offset)
                    segs = [(i0, 0, l0, 512 - l0), (i1, l0, l1, 512 - l1)]
                    sc_pair = []
                    for z in range(2):
                        sc = pp.tile([128, 1024], F32, tag="sc", bufs=2,
                                     name=f"sc{s}_{g}_{p}_{z}")
                        for (i, off, ln, q0) in segs:
                            nc.tensor.matmul(
                                sc[:, off:off + ln],
                                kTd[z * HD:(z + 1) * HD,
                                    i * 128:(i + 1) * 128],
                                qP[g][z * HD:(z + 1) * HD,
                                      s * SB + q0:(s + 1) * SB],
                                start=True, stop=True,
                                tile_position=(z * HD, 0))
                        sc_pair.append(sc)
                    flush_pv()
                    tot = l0 + l1
                    for z in range(2):
                        pt = ptp.tile([128, 1024], BF16, tag="pt",
                                      name=f"pt{s}_{g}_{p}_{z}")
                        nc.scalar.activation(
                            pt[:, 0:tot], sc_pair[z][:, 0:tot],
                            mybir.ActivationFunctionType.Exp,
                            scale=SCALE)
                        if diag:
                            # one 3D-AP multiply masks both micro-diagonals
                            pt2 = pt[:, 0:2 * l0].rearrange(
                                "p (k c) -> p k c", k=2)
                            nc.vector.tensor_tensor(
                                pt2[:, :, 0:128], pt2[:, :, 0:128],
                                tri_sb[:].rearrange("p (o c) -> p o c", o=1)
                                .to_broadcast([128, 2, 128]),
                                mybir.AluOpType.mult)
                        if state["pt0"] is None:
                            state["pt0"] = pt
                        pend.append((pt, segs, z))
                flush_pv()

            def emit_epilogue(s, g, pvs):
                sl = slice(s * SB, (s + 1) * SB)
                for z in range(2):
                    h = 2 * g + z
                    pv = pvs[z]
                    # out_h = pv[0:64] / (S_row + exp(sink_h))
                    srow = epp.tile([1, 512], F32, tag="srow", bufs=2)
                    nc.vector.scalar_tensor_tensor(
                        srow[:], pv[HD:HD + 1, :], es_sb[0:1, h:h + 1],
                        ones512[:], mybir.AluOpType.add,
                        mybir.AluOpType.mult)
                    rrow = epp.tile([1, 512], F32, tag="rrow", bufs=2)
                    nc.vector.reciprocal_approx_fast(rrow[:], srow[:])
                    rbc = epp.tile([HD, 512], F32, tag="rbc", bufs=3)
                    nc.gpsimd.partition_broadcast(rbc[:], rrow[0:1, :])
                    nc.vector.tensor_tensor(
                        oT[h][:, sl], pv[0:HD, :], rbc[:],
                        mybir.AluOpType.mult)
                    # scatter this (head, block) slice into the A2A input
                    nc.sync.dma_start(
                        a2a_in[s][:].rearrange(
                            "(j hh p) n -> hh p j n",
                            j=NCORES, hh=HPC)[h],
                        oT[h][:, sl].rearrange(
                            "p (j n) -> p j n", j=NCORES))
                if g == 1:
                    nc.gpsimd.collective_compute(
                        "AllToAll", mybir.AluOpType.bypass,
                        replica_groups=[list(range(NCORES))],
                        ins=[a2a_in[s].opt()], outs=[a2a_out[s].opt()])

            pvs_live = {}

            def emit_readback(s):
                nc.sync.dma_start(
                    ag_sb[:, :, s * CH:(s + 1) * CH],
                    a2a_out[s][:].rearrange("(o p) n -> p o n", p=128))

            for s in range(NSB):
                # prefetch x^T for block s+2 (bufs=2: waits on s's release)
                if s + 2 < NSB:
                    xts[s + 2] = xtp.tile([128, ND, SB], BF16, tag="xT",
                                          name=f"xT{s + 2}")
                    nc.sync.dma_start(xts[s + 2][:], xT_v[:, s + 2])
                emit_B(s)
                if s > 0:
                    # deferred: previous block's second epilogue lands here
                    # so it doesn't contend with this block's rope on Vector
                    emit_epilogue(s - 1, 1, pvs_live[(s - 1, 1)])
                if s == 1:
                    # wo^T load, data-gated on early attention so it cannot
                    # steal HBM bandwidth from the startup loads
                    nc.gpsimd.tensor_copy(woT_sb[0:1, 0, 0:2],
                                          state["pt0"][0:1, 0:2])
                    nc.gpsimd.dma_start(woT_sb[:], woT_e[:].rearrange(
                        "p (o f) -> p o f", o=ND))
                for g in range(2):
                    warm_burst(6, f"warmc{s}_{g}")
                    pvs = [pp.tile([HD + 1, 512], F32, tag="pv", bufs=2,
                                   name=f"pv{s}_{g}_{z}") for z in range(2)]
                    pvs_live[(s, g)] = pvs
                    emit_C_group(s, g, pvs)
                    if g == 0:
                        emit_epilogue(s, 0, pvs)
                        if s > 0:
                            # readback of block s-1 (its collectives are
                            # long done by mid-C(s))
                            emit_readback(s - 1)
            emit_epilogue(NSB - 1, 1, pvs_live[(NSB - 1, 1)])
            emit_readback(NSB - 1)

            # ---- output projection for my (interleaved) sequence rows ----
            for m in range(MYS // 128):
                # dep-free warm matmuls fill the PE while the last A2A +
                # readback complete, keeping the DVFS clock up
                warm_burst(10, f"warmd{m}")
                for n in range(DIM // 512):
                    fp = pp.tile([128, 512], F32, tag="sc", bufs=2,
                                 name=f"fp{m}_{n}")
                    for kt in range(ND):
                        nc.tensor.matmul(
                            fp[:], ag_sb[:, kt, m * 128:(m + 1) * 128],
                            woT_sb[:, kt, n * 512:(n + 1) * 512],
                            start=(kt == 0), stop=False)
                    nc.tensor.matmul(fp[:], ones_sb[0:1, :],
                                     wob_sb[0:1, n * 512:(n + 1) * 512],
                                     start=False, stop=True)
                    fo = fnp.tile([128, 512], F32, tag="fo")
                    if (m * 4 + n) % 2 == 0:
                        nc.scalar.copy(fo[:], fp[:])
                    else:
                        nc.vector.tensor_copy(fo[:], fp[:])
                    nc.sync.dma_start(
                        out_e[m * 128:(m + 1) * 128,
                              n * 512:(n + 1) * 512], fo[:])

    nc.compile()
    return nc


def _host_prep(x, rope_cache, wq_w, wq_b, wk_w, wk_b, wv_w, wv_b,
               wo_w, wo_b, sinks):
    """Build the per-core input maps (sharding + layout prep)."""
    # x^T, partition-major, grouped so each 512-block is one contiguous DMA:
    # xT[p, s, d, c] = x[512s + c, 128d + p]
    xt = np.asarray(x, np.float32).reshape(S, DIM).T.astype(ml_dtypes.bfloat16)
    xt = np.ascontiguousarray(
        xt.reshape(ND, 128, NSB, SB).transpose(1, 2, 0, 3).reshape(
            128, NSB * ND * SB))

    def _pm(a):
        # [S, F] -> [128, (S//128) * F] partition-major packing
        f = a.shape[1]
        return np.ascontiguousarray(
            a.reshape(S // 128, 128, f).transpose(1, 0, 2).reshape(
                128, (S // 128) * f))

    cos = np.asarray(rope_cache[:, :HD // 2], np.float32)
    sin = np.asarray(rope_cache[:, HD // 2:], np.float32)
    cosd = _pm(np.concatenate([cos, cos], axis=1))
    nsin = _pm(-sin)
    psin = _pm(sin)
    # causal micro-mask: tri[p, j] = 1 iff j >= p
    tri = np.triu(np.ones((128, 128), np.float32)).astype(ml_dtypes.bfloat16)
    ident = np.eye(128, dtype=ml_dtypes.bfloat16)
    woT = np.asarray(wo_w, np.float32).T.astype(ml_dtypes.bfloat16)
    woT = np.ascontiguousarray(
        woT.reshape(ND, 128, DIM).transpose(1, 0, 2).reshape(128, ND * DIM))
    wob = np.asarray(wo_b, np.float32).astype(
        ml_dtypes.bfloat16).reshape(1, DIM)

    in_maps = []
    for c in range(NCORES):
        qsl = slice(c * QW, (c + 1) * QW)
        ksl = slice(c * HD, (c + 1) * HD)
        wqkvT = np.concatenate([
            np.asarray(wq_w, np.float32)[qsl].T,
            np.asarray(wk_w, np.float32)[ksl].T,
            np.asarray(wv_w, np.float32)[ksl].T],
            axis=1).astype(ml_dtypes.bfloat16)
        wqkvT = np.ascontiguousarray(
            wqkvT.reshape(ND, 128, QKV).transpose(1, 0, 2).reshape(
                128, ND * QKV))
        qkvb = np.ascontiguousarray(np.concatenate([
            np.asarray(wq_b, np.float32)[qsl],
            np.asarray(wk_b, np.float32)[ksl],
            np.asarray(wv_b, np.float32)[ksl]]).astype(
                ml_dtypes.bfloat16)).reshape(1, QKV)
        sinks4 = np.ascontiguousarray(
            np.asarray(sinks, np.float32)[c * HPC:(c + 1) * HPC]).reshape(1, HPC)
        in_maps.append({
            "xT": xt, "wqkvT": wqkvT, "qkvb": qkvb, "cosd": cosd,
            "nsin": nsin, "psin": psin, "tri": tri, "ident": ident,
            "woT": woT, "wob": wob, "sinks4": sinks4,
        })
    return in_maps


def kernel(**inputs):
    global last_exec_time_ns, last_result
    if "nc" not in _cache:
        _cache["nc"] = _build()
    nc = _cache["nc"]
    in_maps = _host_prep(**inputs)
    trace = bool(int(os.environ.get("BASS_KERNEL_TRACE", "0")))
    if trace:
        try:
            _install_ntff_shim()
        except Exception:
            trace = False
    tc_env = os.environ.get("BASS_KERNEL_TRACE_CORES")
    kw = {}
    if trace and tc_env:
        kw["trace_cores"] = [int(c) for c in tc_env.split(",")]
    res = run_bass_kernel_spmd(nc, in_maps, core_ids=list(range(NCORES)),
                               trace=trace, **kw)
    last_exec_time_ns = res.exec_time_ns
    last_result = res
    # unshard: core c's row 64s + i  <->  global seq 512s + 64c + i
    out = np.empty((S, NH * HD), np.float32)
    for c in range(NCORES):
        rc = res.results[c]["out"].reshape(NSB, CH, DIM)
        for s in range(NSB):
            out[SB * s + CH * c: SB * s + CH * (c + 1)] = rc[s]
    return out.reshape(B, S, NH * HD)


# revision 23
# speedup vs baseline: 1.0876x; 1.0876x over previous
"""Trainium2 Bass kernel for GQA attention with RoPE, causal mask, and
attention sinks (nn_Attention_65094524338392).

Sharding: tensor-parallel by heads across 8 NeuronCores. Core c owns query
heads 4c..4c+3 and kv-head c (NREP=4). Each core computes QKV projections
over the full sequence for its heads and flash-style causal attention.

v3 design:
- x is transposed on the HOST (free) and streamed as one contiguous DMA
  per 512-token block (no device DMA-transposes). Late-needed loads (woT,
  x blocks 1+) are data-gated so they don't steal HBM bandwidth from the
  startup-critical wqkvT + x-block-0 transfers.
- QKV/rope/attention are interleaved per 512-block: B0 C0 B1 C1 ...
- The head->sequence redistribution is split into 4 per-block AllToAlls
  (sequence ownership interleaved: core c owns rows 512*s + 64c..+64 of
  each block s), each overlapped with later compute; the output
  projection overlaps the last one.
- Scores for the diagonal 128-tiles are trimmed/compacted; the causal
  micro-mask is one [128,128] triangle applied with a single 3D-AP
  vector multiply per k-tile pair.
- Dummy PE warm bursts at attention-group boundaries keep the HAM DVFS
  clock high (same trick as the previous kernel generation).

Math note: the sink scaling folds into the softmax normalizer:
    out = (sum_k exp(s_k) v_k) / (sum_k exp(s_k) + exp(sink))
so no logs/sigmoids are needed on device, and because |s| <= ~40 no
max-subtraction is needed for exp stability in fp32 accumulation.
"""

import os
import sys

sys.path.insert(0, "/opt/trn_rl_repo")

import ml_dtypes
import numpy as np

import concourse.bass as bass
import concourse.mybir as mybir
import concourse.tile as tile
from concourse import bacc
from concourse.bass_utils import run_bass_kernel_spmd

# Problem shapes
B, S, DIM = 1, 2048, 2048
NH, NKV, HD = 32, 8, 64
NREP = NH // NKV
SCALE = 1.0 / float(np.sqrt(HD))
NCORES = 8
HPC = NH // NCORES            # query heads per core (4)
QKV = HPC * HD + 2 * HD       # fused qkv output dim per core (384)
QW = HPC * HD                 # query width per core (256)
SB = 512                      # seq block
NSB = S // SB                 # 4
NT = S // 128                 # 16 seq tiles
ND = DIM // 128               # 16 contraction tiles
MYS = S // NCORES             # output rows per core (256)
CH = MYS // NSB               # rows per core per block (64)

F32 = mybir.dt.float32
BF16 = mybir.dt.bfloat16

_cache = {}

last_exec_time_ns = None
last_result = None


def _install_ntff_shim():
    """Register the NTFF profile hook so trace=True yields exec_time_ns."""
    import types
    if "antenv.axon_hooks" in sys.modules:
        return
    import antenv
    mod = types.ModuleType("antenv.axon_hooks")
    mod._hook = None
    mod.set_axon_ntff_profile_hook = lambda h: setattr(mod, "_hook", h)
    mod.get_axon_ntff_profile_hook = lambda: mod._hook
    sys.modules["antenv.axon_hooks"] = mod
    antenv.axon_hooks = mod
    from trn_agent_boot.trn_boot import _ntff_profile_via_ctypes
    hook = _ntff_profile_via_ctypes("/opt/axon/libaxon_pjrt.so")
    if hook is not None:
        mod._hook = hook


def _build():
    nc = bacc.Bacc("TRN2", target_bir_lowering=False, debug=False,
                   num_devices=NCORES)

    ident_e = nc.declare_dram_parameter("ident", [128, 128], BF16, isOutput=False)
    qkvb_e = nc.declare_dram_parameter("qkvb", [1, QKV], BF16, isOutput=False)
    cosd_e = nc.declare_dram_parameter("cosd", [128, NT * HD], F32, isOutput=False)
    nsin_e = nc.declare_dram_parameter("nsin", [128, NT * HD // 2], F32, isOutput=False)
    psin_e = nc.declare_dram_parameter("psin", [128, NT * HD // 2], F32, isOutput=False)
    sinks_e = nc.declare_dram_parameter("sinks4", [1, HPC], F32, isOutput=False)
    wob_e = nc.declare_dram_parameter("wob", [1, DIM], BF16, isOutput=False)
    tri_e = nc.declare_dram_parameter("tri", [128, 128], BF16, isOutput=False)
    wqkvT_e = nc.declare_dram_parameter("wqkvT", [128, ND * QKV], BF16, isOutput=False)
    xT_e = nc.declare_dram_parameter("xT", [128, NSB * ND * SB], BF16, isOutput=False)
    woT_e = nc.declare_dram_parameter("woT", [128, ND * DIM], BF16, isOutput=False)
    out_e = nc.declare_dram_parameter("out", [MYS, DIM], F32, isOutput=True)

    xT_v = xT_e[:].rearrange("p (s d c) -> p s d c", s=NSB, d=ND)

    with tile.TileContext(nc) as tc:
        with tc.tile_pool(name="const", bufs=1) as cp, \
             tc.tile_pool(name="xT", bufs=2) as xtp, \
             tc.tile_pool(name="rope", bufs=3) as rp, \
             tc.tile_pool(name="qkr", bufs=5) as qkrp, \
             tc.tile_pool(name="pt", bufs=4) as ptp, \
             tc.tile_pool(name="ep", bufs=2) as epp, \
             tc.tile_pool(name="fin", bufs=3) as fnp, \
             tc.tile_pool(name="dram", bufs=1, space="DRAM") as dp, \
             tc.tile_pool(name="pp", bufs=2, space="PSUM") as pp:

            # ---- startup ----
            # gpsimd: memsets first (instant; unblock warmup + bias rows)
            ones_sb = cp.tile([1, 128], BF16)
            nc.gpsimd.memset(ones_sb[:], 1.0)
            ones512 = cp.tile([1, 512], BF16)
            nc.gpsimd.memset(ones512[:], 1.0)
            warm_sb = cp.tile([128, 512], BF16)
            nc.gpsimd.memset(warm_sb[:], 0.0)
            # sync queue: wqkvT then x^T block 0 (the startup-critical loads)
            wqkvT_sb = cp.tile([128, ND, QKV], BF16)
            nc.sync.dma_start(wqkvT_sb[:], wqkvT_e[:].rearrange(
                "p (o f) -> p o f", o=ND))
            xts = [None] * NSB
            xts[0] = xtp.tile([128, ND, SB], BF16, tag="xT", name="xT0")
            nc.sync.dma_start(xts[0][:], xT_v[:, 0])
            # scalar queue (HWDGE): tiny consts + rope tables
            ident_sb = cp.tile([128, 128], BF16)
            nc.scalar.dma_start(ident_sb[:], ident_e[:])
            qkvb_sb = cp.tile([1, QKV], BF16)
            nc.scalar.dma_start(qkvb_sb[:], qkvb_e[:])
            tri_sb = cp.tile([128, 128], BF16)
            nc.scalar.dma_start(tri_sb[:], tri_e[:])
            sinks_sb = cp.tile([1, HPC], F32)
            nc.scalar.dma_start(sinks_sb[:], sinks_e[:])
            wob_sb = cp.tile([1, DIM], BF16)
            nc.scalar.dma_start(wob_sb[:], wob_e[:])
            # warm the ACT exp table during startup
            es_sb = cp.tile([1, HPC], F32)
            nc.scalar.activation(es_sb[:], sinks_sb[:],
                                 mybir.ActivationFunctionType.Exp)
            cos_sb = cp.tile([128, NT, HD], F32)
            nc.scalar.dma_start(cos_sb[:], cosd_e[:].rearrange(
                "p (o f) -> p o f", o=NT))
            nsin_sb = cp.tile([128, NT, HD // 2], F32)
            nc.scalar.dma_start(nsin_sb[:], nsin_e[:].rearrange(
                "p (o f) -> p o f", o=NT))
            psin_sb = cp.tile([128, NT, HD // 2], F32)
            nc.scalar.dma_start(psin_sb[:], psin_e[:].rearrange(
                "p (o f) -> p o f", o=NT))
            # x^T block 1: gated on the rope tables' arrival (the last
            # startup-critical load) so its 2MB doesn't steal HBM bandwidth
            # from them; B1 needs it much later
            xts[1] = xtp.tile([128, ND, SB], BF16, tag="xT", name="xT1")
            nc.gpsimd.tensor_copy(xts[1][0:1, 0, 0:2],
                                  psin_sb[0:1, NT - 1, HD // 2 - 2:HD // 2])
            nc.sync.dma_start(xts[1][:], xT_v[:, 1])

            warm2_sb = cp.tile([128, 512], BF16)
            nc.gpsimd.memset(warm2_sb[:], 0.0)

            def warm_burst(n, name, src_sb=None):
                ws = warm_sb if src_sb is None else src_sb
                w = pp.tile([128, 512], F32, tag="acc", bufs=2, name=name)
                for _ in range(n):
                    nc.tensor.matmul(w[:, 0:512], ws[:, 0:128],
                                     ws[:], start=True, stop=True)

            # PE warm-up during the initial input DMAs (HAM clock release)
            warm_burst(16, "warm0")
            woT_sb = cp.tile([128, ND, DIM], BF16)

            # persistent activations
            qP = [cp.tile([128, S], BF16, name=f"qP{g}") for g in range(HPC // 2)]
            kTd = cp.tile([128, S], BF16)
            v_sb = cp.tile([128, NT, HD + 1], BF16)
            nc.gpsimd.memset(v_sb[:, :, HD:HD + 1], 1.0)
            oT = [cp.tile([HD, S], BF16, name=f"oT{h}") for h in range(HPC)]
            ag_sb = cp.tile([128, ND, MYS], BF16)

            a2a_in = [dp.tile([NH * HD, CH], BF16, name=f"a2ain{s}")
                      for s in range(NSB)]
            a2a_out = [tc.tile([NH * HD, CH], BF16, space="DRAM",
                               addr_space="Shared", name=f"a2aout{s}")[0]
                       for s in range(NSB)]

            state = {"pt0": None}

            def emit_B(s):
                xT_s = xts[s]
                qkr_tiles = []
                for tt in range(4):
                    t = 4 * s + tt
                    acc = pp.tile([128, 512], F32, tag="acc", bufs=2,
                                  name=f"acc{t}")
                    for d in range(ND):
                        nc.tensor.matmul(acc[:, 0:QKV],
                                         xT_s[:, d, tt * 128:(tt + 1) * 128],
                                         wqkvT_sb[:, d, :],
                                         start=(d == 0), stop=False)
                    nc.tensor.matmul(acc[:, 0:QKV], ones_sb[0:1, :], qkvb_sb[:],
                                     start=False, stop=True)

                    # rope on q and k (free-dim halves; 5 = 4q + 1k groups)
                    W = QW + HD  # 320
                    tmp = rp.tile([128, W], F32, tag="tmp")
                    qkr = qkrp.tile([128, W], BF16, tag="qkr")
                    acc5 = acc[:, 0:W].rearrange("p (h x) -> p h x", x=HD)
                    tmp5 = tmp[:].rearrange("p (h x) -> p h x", x=HD)
                    nc.vector.tensor_tensor(
                        tmp5[:, :, 0:HD // 2], acc5[:, :, HD // 2:HD],
                        nsin_sb[:, t:t + 1, :].to_broadcast([128, 5, HD // 2]),
                        mybir.AluOpType.mult)
                    nc.vector.tensor_tensor(
                        tmp5[:, :, HD // 2:HD], acc5[:, :, 0:HD // 2],
                        psin_sb[:, t:t + 1, :].to_broadcast([128, 5, HD // 2]),
                        mybir.AluOpType.mult)
                    nc.vector.tensor_tensor(
                        qkr[:], acc[:, 0:W],
                        cos_sb[:, t:t + 1, :].to_broadcast([128, 5, HD]),
                        mybir.AluOpType.mult)
                    nc.vector.tensor_tensor(qkr[:], qkr[:], tmp[:],
                                            mybir.AluOpType.add)
                    qkr_tiles.append(qkr)
                    # v: plain copy (cast to bf16)
                    nc.scalar.copy(v_sb[:, t, 0:HD], acc[:, QW + HD:QKV])

                # transpose rope'd q/k into [hd, seq] layout
                sl = slice(s * SB, (s + 1) * SB)
                for h in range(HPC + 1):
                    tq_ps = pp.tile([HD, 512], BF16, tag="sc", bufs=2,
                                    name=f"tq{s}_{h}")
                    for tt in range(4):
                        nc.tensor.transpose(
                            tq_ps[:, tt * 128:(tt + 1) * 128],
                            qkr_tiles[tt][:, h * HD:(h + 1) * HD], ident_sb[:])
                    if h < HPC:
                        dst = qP[h // 2][(h % 2) * HD:(h % 2) * HD + HD, sl]
                        if h % 2 == 0:
                            nc.vector.tensor_copy(dst, tq_ps[:])
                        else:
                            nc.scalar.copy(dst, tq_ps[:])
                    else:
                        nc.scalar.copy(kTd[0:HD, sl], tq_ps[:])
                        nc.vector.tensor_copy(kTd[HD:2 * HD, sl], tq_ps[:])

            def emit_C_group(s, g, pvs):
                n_kt = 4 * (s + 1)
                pend = []

                def flush_pv():
                    for (pts_, segs_, z_) in pend:
                        for (i, off, ln, q0) in segs_:
                            nc.tensor.matmul(
                                pvs[z_][:, q0:q0 + ln],
                                v_sb[:, i, :], pts_[:, off:off + ln],
                                start=(i == 0), stop=(i == n_kt - 1),
                                skip_group_check=True)
                    pend.clear()

                for p in range(n_kt // 2):
                    i0, i1 = 2 * p, 2 * p + 1
                    diag = (i0 >= 4 * s)
                    if diag:
                        d0, d1 = i0 - 4 * s, i1 - 4 * s
                        l0, l1 = 512 - 128 * d0, 512 - 128 * d1
                    else:
                        l0 = l1 = 512
                    # segs: (ktile, col offset in sc/pt, len, q offset)
                    segs = [(i0, 0, l0, 512 - l0), (i1, l0, l1, 512 - l1)]
                    sc_pair = []
                    for z in range(2):
                        sc = pp.tile([128, 1024], F32, tag="sc", bufs=2,
                                     name=f"sc{s}_{g}_{p}_{z}")
                        for (i, off, ln, q0) in segs:
                            nc.tensor.matmul(
                                sc[:, off:off + ln],
                                kTd[z * HD:(z + 1) * HD,
                                    i * 128:(i + 1) * 128],
                                qP[g][z * HD:(z + 1) * HD,
                                      s * SB + q0:(s + 1) * SB],
                                start=True, stop=True,
                                tile_position=(z * HD, 0))
                        sc_pair.append(sc)
                    flush_pv()
                    tot = l0 + l1
                    for z in range(2):
                        pt = ptp.tile([128, 1024], BF16, tag="pt",
                                      name=f"pt{s}_{g}_{p}_{z}")
                        nc.scalar.activation(
                            pt[:, 0:tot], sc_pair[z][:, 0:tot],
                            mybir.ActivationFunctionType.Exp,
                            scale=SCALE)
                        if diag:
                            # one 3D-AP multiply masks both micro-diagonals
                            pt2 = pt[:, 0:2 * l0].rearrange(
                                "p (k c) -> p k c", k=2)
                            nc.vector.tensor_tensor(
                                pt2[:, :, 0:128], pt2[:, :, 0:128],
                                tri_sb[:].rearrange("p (o c) -> p o c", o=1)
                                .to_broadcast([128, 2, 128]),
                                mybir.AluOpType.mult)
                        if state["pt0"] is None:
                            state["pt0"] = pt
                        pend.append((pt, segs, z))
                flush_pv()

            def emit_epilogue(s, g, pvs):
                sl = slice(s * SB, (s + 1) * SB)
                for z in range(2):
                    h = 2 * g + z
                    pv = pvs[z]
                    # out_h = pv[0:64] / (S_row + exp(sink_h))
                    srow = epp.tile([1, 512], F32, tag="srow", bufs=2)
                    nc.vector.scalar_tensor_tensor(
                        srow[:], pv[HD:HD + 1, :], es_sb[0:1, h:h + 1],
                        ones512[:], mybir.AluOpType.add,
                        mybir.AluOpType.mult)
                    rrow = epp.tile([1, 512], F32, tag="rrow", bufs=2)
                    nc.vector.reciprocal_approx_fast(rrow[:], srow[:])
                    rbc = epp.tile([HD, 512], F32, tag="rbc", bufs=3)
                    nc.gpsimd.partition_broadcast(rbc[:], rrow[0:1, :])
                    nc.vector.tensor_tensor(
                        oT[h][:, sl], pv[0:HD, :], rbc[:],
                        mybir.AluOpType.mult)
                    # scatter this (head, block) slice into the A2A input
                    nc.sync.dma_start(
                        a2a_in[s][:].rearrange(
                            "(j hh p) n -> hh p j n",
                            j=NCORES, hh=HPC)[h],
                        oT[h][:, sl].rearrange(
                            "p (j n) -> p j n", j=NCORES))
                if g == 1:
                    nc.gpsimd.collective_compute(
                        "AllToAll", mybir.AluOpType.bypass,
                        replica_groups=[list(range(NCORES))],
                        ins=[a2a_in[s].opt()], outs=[a2a_out[s].opt()])

            pvs_live = {}

            def emit_readback(s):
                nc.sync.dma_start(
                    ag_sb[:, :, s * CH:(s + 1) * CH],
                    a2a_out[s][:].rearrange("(o p) n -> p o n", p=128))

            for s in range(NSB):
                # prefetch x^T for block s+2 (bufs=2: waits on s's release)
                if s + 2 < NSB:
                    xts[s + 2] = xtp.tile([128, ND, SB], BF16, tag="xT",
                                          name=f"xT{s + 2}")
                    nc.sync.dma_start(xts[s + 2][:], xT_v[:, s + 2])
                emit_B(s)
                if s > 0:
                    # deferred: previous block's second epilogue lands here
                    # so it doesn't contend with this block's rope on Vector
                    emit_epilogue(s - 1, 1, pvs_live[(s - 1, 1)])
                if s == 1:
                    # wo^T load, data-gated on early attention so it cannot
                    # steal HBM bandwidth from the startup loads
                    nc.gpsimd.tensor_copy(woT_sb[0:1, 0, 0:2],
                                          state["pt0"][0:1, 0:2])
                    nc.gpsimd.dma_start(woT_sb[:], woT_e[:].rearrange(
                        "p (o f) -> p o f", o=ND))
                for g in range(2):
                    warm_burst(6, f"warmc{s}_{g}")
                    pvs = [pp.tile([HD + 1, 512], F32, tag="pv", bufs=2,
                                   name=f"pv{s}_{g}_{z}") for z in range(2)]
                    pvs_live[(s, g)] = pvs
                    emit_C_group(s, g, pvs)
                    if g == 0:
                        emit_epilogue(s, 0, pvs)
                        if s > 0:
                            # readback of block s-1 (its collectives are
                            # long done by mid-C(s))
                            emit_readback(s - 1)
            emit_epilogue(NSB - 1, 1, pvs_live[(NSB - 1, 1)])
            emit_readback(NSB - 1)
            # data-gated warm tile: becomes ready only once the last block's
            # output is written, so these warm matmuls fill the PE during
            # the final A2A + readback instead of running early
            nc.vector.tensor_copy(warm2_sb[0:1, 0:4],
                                  oT[HPC - 1][0:1, S - 4:S])

            # ---- output projection for my (interleaved) sequence rows ----
            for m in range(MYS // 128):
                # dep-free warm matmuls fill the PE while the last A2A +
                # readback complete, keeping the DVFS clock up
                warm_burst(10, f"warmd{m}", src_sb=(warm2_sb if m else None))
                for n in range(DIM // 512):
                    fp = pp.tile([128, 512], F32, tag="sc", bufs=2,
                                 name=f"fp{m}_{n}")
                    for kt in range(ND):
                        nc.tensor.matmul(
                            fp[:], ag_sb[:, kt, m * 128:(m + 1) * 128],
                            woT_sb[:, kt, n * 512:(n + 1) * 512],
                            start=(kt == 0), stop=False)
                    nc.tensor.matmul(fp[:], ones_sb[0:1, :],
                                     wob_sb[0:1, n * 512:(n + 1) * 512],
                                     start=False, stop=True)
                    fo = fnp.tile([128, 512], F32, tag="fo")
                    if (m * 4 + n) % 2 == 0:
                        nc.scalar.copy(fo[:], fp[:])
                    else:
                        nc.vector.tensor_copy(fo[:], fp[:])
                    nc.sync.dma_start(
                        out_e[m * 128:(m + 1) * 128,
                              n * 512:(n + 1) * 512], fo[:])

    nc.compile()
    return nc


def _host_prep(x, rope_cache, wq_w, wq_b, wk_w, wk_b, wv_w, wv_b,
               wo_w, wo_b, sinks):
    """Build the per-core input maps (sharding + layout prep)."""
    # x^T, partition-major, grouped so each 512-block is one contiguous DMA:
    # xT[p, s, d, c] = x[512s + c, 128d + p]
    xt = np.asarray(x, np.float32).reshape(S, DIM).T.astype(ml_dtypes.bfloat16)
    xt = np.ascontiguousarray(
        xt.reshape(ND, 128, NSB, SB).transpose(1, 2, 0, 3).reshape(
            128, NSB * ND * SB))

    def _pm(a):
        # [S, F] -> [128, (S//128) * F] partition-major packing
        f = a.shape[1]
        return np.ascontiguousarray(
            a.reshape(S // 128, 128, f).transpose(1, 0, 2).reshape(
                128, (S // 128) * f))

    cos = np.asarray(rope_cache[:, :HD // 2], np.float32)
    sin = np.asarray(rope_cache[:, HD // 2:], np.float32)
    cosd = _pm(np.concatenate([cos, cos], axis=1))
    nsin = _pm(-sin)
    psin = _pm(sin)
    # causal micro-mask: tri[p, j] = 1 iff j >= p
    tri = np.triu(np.ones((128, 128), np.float32)).astype(ml_dtypes.bfloat16)
    ident = np.eye(128, dtype=ml_dtypes.bfloat16)
    woT = np.asarray(wo_w, np.float32).T.astype(ml_dtypes.bfloat16)
    woT = np.ascontiguousarray(
        woT.reshape(ND, 128, DIM).transpose(1, 0, 2).reshape(128, ND * DIM))
    wob = np.asarray(wo_b, np.float32).astype(
        ml_dtypes.bfloat16).reshape(1, DIM)

    in_maps = []
    for c in range(NCORES):
        qsl = slice(c * QW, (c + 1) * QW)
        ksl = slice(c * HD, (c + 1) * HD)
        wqkvT = np.concatenate([
            np.asarray(wq_w, np.float32)[qsl].T,
            np.asarray(wk_w, np.float32)[ksl].T,
            np.asarray(wv_w, np.float32)[ksl].T],
            axis=1).astype(ml_dtypes.bfloat16)
        wqkvT = np.ascontiguousarray(
            wqkvT.reshape(ND, 128, QKV).transpose(1, 0, 2).reshape(
                128, ND * QKV))
        qkvb = np.ascontiguousarray(np.concatenate([
            np.asarray(wq_b, np.float32)[qsl],
            np.asarray(wk_b, np.float32)[ksl],
            np.asarray(wv_b, np.float32)[ksl]]).astype(
                ml_dtypes.bfloat16)).reshape(1, QKV)
        sinks4 = np.ascontiguousarray(
            np.asarray(sinks, np.float32)[c * HPC:(c + 1) * HPC]).reshape(1, HPC)
        in_maps.append({
            "xT": xt, "wqkvT": wqkvT, "qkvb": qkvb, "cosd": cosd,
            "nsin": nsin, "psin": psin, "tri": tri, "ident": ident,
            "woT": woT, "wob": wob, "sinks4": sinks4,
        })
    return in_maps


def kernel(**inputs):
    global last_exec_time_ns, last_result
    if "nc" not in _cache:
        _cache["nc"] = _build()
    nc = _cache["nc"]
    in_maps = _host_prep(**inputs)
    trace = bool(int(os.environ.get("BASS_KERNEL_TRACE", "0")))
    if trace:
        try:
            _install_ntff_shim()
        except Exception:
            trace = False
    tc_env = os.environ.get("BASS_KERNEL_TRACE_CORES")
    kw = {}
    if trace and tc_env:
        kw["trace_cores"] = [int(c) for c in tc_env.split(",")]
    res = run_bass_kernel_spmd(nc, in_maps, core_ids=list(range(NCORES)),
                               trace=trace, **kw)
    last_exec_time_ns = res.exec_time_ns
    last_result = res
    # unshard: core c's row 64s + i  <->  global seq 512s + 64c + i
    out = np.empty((S, NH * HD), np.float32)
    for c in range(NCORES):
        rc = res.results[c]["out"].reshape(NSB, CH, DIM)
        for s in range(NSB):
            out[SB * s + CH * c: SB * s + CH * (c + 1)] = rc[s]
    return out.reshape(B, S, NH * HD)
